# revision 1
# baseline (speedup 1.0000x reference)
"""Trainium2 Bass kernel for nn_AttentionProbe_80891414053184.

Math (reference):
    y  = relu(x @ W1.T + b1)            # (B,S,H) -> (B,S,128)
    y2 = relu(y @ W2.T + b2)            # (B,S,128)
    l  = y2 @ Wq.T + pos*pos_w  (+mask) # (B,S,8) logits
    p  = softmax(l, axis=S)
    v  = y2 @ Wv.T + bv
    out[b] = sum_{s,h} p*v + bias       # (B,1)

Strategy: sequence-parallel over 8 cores (512 positions x 4 batches = 2048
tokens per core).  Each core streams its x-shard (pre-transposed on host to
(H, tokens) so the contraction dim lands on SBUF partitions), runs the MLP +
head projections on-chip, and emits per-(batch, head) partial softmax stats
(-max, Z=sum exp, W=sum exp*v).  The host merges the 8 partial stats with the
standard online-softmax combine and produces the (4,1) output.
"""

import os

import numpy as np

# Problem dims (hardcoded per harness contract).
B, S, H = 4, 4096, 4096
MLP, NH = 128, 8
NCORES = 8
S_SHARD = S // NCORES        # 512 seq positions per core
TOK = B * S_SHARD            # 2048 tokens per core
NT = TOK // 512              # 4 token tiles of 512 (= one batch each)
KCH = H // 128               # 32 contraction chunks

# Layer-1 operand dtype: bf16 halves HBM traffic for x AND runs the PE at
# 1 cycle/row (fp32 takes 4, f32r ~2-3).  Measured end-to-end output error of
# the bf16 path vs the fp32 reference is ~2e-4 (softmax averaging washes out
# the rounding noise).  Override with KERNEL_DT=f32|f32r|bf16 for A/B tests.
KERNEL_DT = os.environ.get("KERNEL_DT",
                           "f32r" if os.environ.get("KERNEL_F32R") == "1"
                           else "bf16")

_cache = {}


def _build_nc(h, dt_name):
    import concourse.mybir as mybir
    import concourse.tile as tile
    from concourse import bacc
    from concourse.tile import add_dep_helper

    f32 = mybir.dt.float32
    f32r = mybir.dt.float32r
    fmm = {"f32": f32, "f32r": f32r,
           "bf16": mybir.dt.bfloat16}[dt_name]
    kch = h // 128

    # Bacc (not bare Bass): its finalize() runs move_matmul_waits_to_ldweights
    # and generate_event_semaphores, which split multi-sem waits to satisfy
    # TRN2's one-wait-per-instruction encoding limit.
    nc = bacc.Bacc()
    xt_d = nc.dram_tensor("xt", [h, TOK], fmm, kind="ExternalInput")
    w1_d = nc.dram_tensor("w1s", [128, kch, MLP], fmm, kind="ExternalInput")
    # cw: [w2t | wq32 (4 x 32-wide zero-padded blocks) | wv32 | b1 | b2] on 128
    # partitions; ca: [addt32 | bv32] on 32 partitions (row 8t+h = tile t,
    # head h).  The zero-padded head blocks let all 4 token tiles accumulate
    # their head projections into ONE (32, 512) psum bank, so the whole
    # softmax-stats stage runs as single 32-lane instructions.
    P32 = NT * NH               # 32 packed (tile, head) lanes
    f32r = mybir.dt.float32r
    # cwr: same [w2t | wq32 | wv32] payload typed float32r — the tail matmuls
    # read it at the fast PE rate (measured f32r accuracy on this net: 1.4e-5)
    cwr_d = nc.dram_tensor("cwr", [MLP, MLP + 2 * P32 * NT], f32r,
                           kind="ExternalInput")
    cw_d = nc.dram_tensor("cw", [MLP, MLP + 2 * P32 * NT + 2], f32,
                          kind="ExternalInput")
    ca_d = nc.dram_tensor("ca", [P32, 512 + 1], f32, kind="ExternalInput")
    st_d = nc.dram_tensor("stats", [P32, 3], f32, kind="ExternalOutput")

    AF = mybir.ActivationFunctionType
    AX = mybir.AxisListType
    OP = mybir.AluOpType
    CQ = MLP                    # wq32 blocks start
    CV = MLP + P32 * NT         # wv32 blocks start
    CB1 = MLP + 2 * P32 * NT    # col index of b1 in cw

    with tile.TileContext(nc) as tc:
        GRP = 2                      # k-chunks per x DMA (1 MB bf16 transfers)
        with (
            tc.tile_pool(name="const", bufs=1) as const,
            tc.tile_pool(name="xp", bufs=8) as xp,
            tc.tile_pool(name="yp", bufs=4) as yp,
            tc.tile_pool(name="y2p", bufs=4) as y2p,
            tc.tile_pool(name="smallp", bufs=1) as smallp,
            tc.tile_pool(name="statsp", bufs=1) as statsp,
            tc.tile_pool(name="ps_y", bufs=4, space="PSUM") as ps_y,
            tc.tile_pool(name="ps_y2", bufs=2, space="PSUM") as ps_y2,
            tc.tile_pool(name="ps_q", bufs=1, space="PSUM") as ps_q,
            tc.tile_pool(name="ps_v", bufs=1, space="PSUM") as ps_v,
        ):
            w1_sb = const.tile([128, kch, MLP], fmm)
            # halves: the first real matmul only needs chunk 0, so the gating
            # transfer is half of w1
            h2 = kch // 2
            nc.sync.dma_start(out=w1_sb[:, 0:h2, :], in_=w1_d[:, 0:h2, :])
            nc.sync.dma_start(out=w1_sb[:, h2:kch, :], in_=w1_d[:, h2:kch, :])
            ca_sb = const.tile([P32, 512 + 1], f32)
            nc.sync.dma_start(out=ca_sb[:], in_=ca_d[:])
            cw_sb = const.tile([MLP, MLP + 2 * P32 * NT + 2], f32)
            nc.sync.dma_start(out=cw_sb[:], in_=cw_d[:])
            cwr_sb = const.tile([MLP, MLP + 2 * P32 * NT], f32r)
            nc.sync.dma_start(out=cwr_sb[:], in_=cwr_d[:])

            stats_sb = statsp.tile([P32, 3], f32)

            # --- Warmup / staging: each engine observes every const-DMA lane
            # once, so steady-state instructions carry at most one new wait
            # (fewer split-events from Bacc's generate_event_semaphores).
            # Only the w1 warmup gates the k-loop; cw/ca warms run after it so
            # the first real matmul waits on nothing but w1-half0 + x-group0.
            warm_ps = ps_y2.tile([128, 512], f32, tag="y2", name="warm_ps")
            warm_pe_last = nc.tensor.matmul(warm_ps[:, 0:NH], w1_sb[:, 0, :],
                                            w1_sb[:, 0, 0:NH],
                                            start=True, stop=True)

            # Layer 1: yT[t] (128, 512) += W1T_chunk.T @ xT_chunk, k-accumulated
            psum_y = []
            for t in range(NT):
                y_ps = ps_y.tile([128, 512], f32, tag="y", name=f"y_ps{t}")
                psum_y.append(y_ps)

            # A recycled x-slot DMA needs a PE wait (WAR on the slot's last
            # matmul reader); park it on a tiny Pool escort op so the DMA
            # itself keeps a single wait (cheap even post-Bacc).
            mm_last = {}
            ngrp = kch // GRP
            for g in range(ngrp - 1):
                x_sb = xp.tile([128, GRP, TOK], fmm, tag="x", name=f"x_sb{g}")
                dma = nc.gpsimd.dma_start(
                    out=x_sb[:],
                    in_=xt_d[g * GRP * 128:(g + 1) * GRP * 128, :].rearrange(
                        "(a p) n -> p a n", p=128))
                if g >= 4:   # pace DMA a few groups ahead of PE consumption
                    esc_t = const.tile([1, 1], f32, name=f"esc_x{g}")
                    esc = nc.gpsimd.memset(esc_t[:], 0.0)
                    add_dep_helper(esc.ins, mm_last[g - 4].ins, sync=True,
                                   reason="escort PE wait for x slot WAR")
                    add_dep_helper(dma.ins, esc.ins, sync=False,
                                   reason="escort precedes dma")
                for kk in range(GRP):
                    k = g * GRP + kk
                    for t in range(NT):
                        mm = nc.tensor.matmul(
                            psum_y[t][:],
                            w1_sb[:, k, :],
                            x_sb[:, kk, t * 512:(t + 1) * 512],
                            start=(k == 0),
                            stop=(k == kch - 1),
                        )
                        if k == 0 and t == 0:
                            add_dep_helper(mm.ins, warm_pe_last.ins,
                                           sync=False,
                                           reason="warmups before first mm")
                mm_last[g] = mm

            # Last k-group arrives per token tile so tile t's MLP tail starts
            # while tile t+1's final columns are still streaming.
            gl = ngrp - 1
            for t in range(NT):
                xl_sb = xp.tile([128, GRP, 512], fmm, tag="xl", name=f"xl{t}", bufs=4)
                nc.gpsimd.dma_start(
                    out=xl_sb[:],
                    in_=xt_d[gl * GRP * 128:(gl + 1) * GRP * 128,
                             t * 512:(t + 1) * 512].rearrange(
                                 "(a p) n -> p a n", p=128))
                for kk in range(GRP):
                    k = gl * GRP + kk
                    nc.tensor.matmul(psum_y[t][:], w1_sb[:, k, :],
                                     xl_sb[:, kk, :],
                                     start=False, stop=(k == kch - 1))

            # cw/ca lane warmups (before their first real consumers in the tail)
            warm_ps2 = ps_y2.tile([128, 512], f32, tag="y2", name="warm_ps2")
            nc.tensor.matmul(warm_ps2[0:NH, 0:NH], cwr_sb[:, 0:NH],
                             cwr_sb[:, 0:NH], start=True, stop=True)
            warm_act = const.tile([MLP, 1], f32)
            nc.scalar.copy(out=warm_act[:], in_=cw_sb[:, CB1:CB1 + 1])
            warm_act8 = const.tile([P32, 1], f32)
            nc.scalar.copy(out=warm_act8[:], in_=ca_sb[:, 512:513])
            warm_dve = const.tile([P32, 1], f32)
            nc.vector.tensor_copy(out=warm_dve[:], in_=ca_sb[:, 0:1])

            q32_ps = ps_q.tile([P32, 512], f32, tag="q", name="q32_ps")
            v32_ps = ps_v.tile([P32, 512], f32, tag="v", name="v32_ps")
            for t in range(NT):
                y_sb = yp.tile([128, 512], f32r, tag="ysb", name=f"y_sb{t}")
                # relu on DVE (add+max) keeps ACT free for relu2/exp in the tail
                nc.vector.tensor_scalar(out=y_sb[:], in0=psum_y[t][:],
                                        scalar1=cw_sb[:, CB1:CB1 + 1],
                                        scalar2=0.0, op0=OP.add, op1=OP.max)
                y2_ps = ps_y2.tile([128, 512], f32, tag="y2", name=f"y2_ps{t}")
                nc.tensor.matmul(y2_ps[:], cwr_sb[:, 0:MLP], y_sb[:],
                                 start=True, stop=True)
                y2_sb = y2p.tile([128, 512], f32r, tag="y2sb", name=f"y2_sb{t}")
                nc.scalar.activation(out=y2_sb[:], in_=y2_ps[:], func=AF.Relu,
                                     bias=cw_sb[:, CB1 + 1:CB1 + 2], scale=1.0)
                # Head projections: the (128, 32) weight block for tile t is
                # zero outside rows 8t..8t+8, so accumulating all 4 tiles into
                # one (32, 512) bank packs q/v as (tile, head) x seq lanes.
                nc.tensor.matmul(q32_ps[:], cwr_sb[:, CQ + P32 * t:CQ + P32 * (t + 1)],
                                 y2_sb[:], start=(t == 0), stop=(t == NT - 1))
                nc.tensor.matmul(v32_ps[:], cwr_sb[:, CV + P32 * t:CV + P32 * (t + 1)],
                                 y2_sb[:], start=(t == 0), stop=(t == NT - 1))

            l_sb = smallp.tile([P32, 512], f32, tag="l", name="l_sb")
            nc.vector.tensor_add(out=l_sb[:], in0=q32_ps[:], in1=ca_sb[:, 0:512])
            # stats[:, 0] = -max_s l
            nc.vector.tensor_reduce(out=stats_sb[:, 0:1], in_=l_sb[:],
                                    axis=AX.X, op=OP.max, negate=True)
            v_sb = smallp.tile([P32, 512], f32, tag="vs", name="v_sb")
            nc.vector.tensor_scalar_add(out=v_sb[:], in0=v32_ps[:],
                                        scalar1=ca_sb[:, 512:513])
            e_sb = smallp.tile([P32, 512], f32, tag="e", name="e_sb")
            # e = exp(l - max); stats[:, 1] = Z = sum e
            nc.scalar.activation(out=e_sb[:], in_=l_sb[:], func=AF.Exp,
                                 bias=stats_sb[:, 0:1], scale=1.0,
                                 accum_out=stats_sb[:, 1:2])
            ev_sb = smallp.tile([P32, 512], f32, tag="ev", name="ev_sb")
            nc.vector.tensor_mul(out=ev_sb[:], in0=e_sb[:], in1=v_sb[:])
            # stats[:, 2] = W = sum e*v
            nc.vector.tensor_reduce(out=stats_sb[:, 2:3], in_=ev_sb[:],
                                    axis=AX.X, op=OP.add)

            nc.gpsimd.dma_start(out=st_d[:], in_=stats_sb[:])

    nc.finalize()
    return nc


def get_nc(h=H, dt_name=None, use_f32r=None):
    if dt_name is None:
        dt_name = ("f32r" if use_f32r else "f32") if use_f32r is not None \
            else KERNEL_DT
    key = (h, dt_name)
    if key not in _cache:
        _cache[key] = _build_nc(h, dt_name)
    return _cache[key]


def make_core_inputs(x, mask, W1, b1, W2, b2, Wq, Wv, bv, pos_w, bias):
    """Host-side shard + transpose. Returns list of 8 in_maps."""
    h = x.shape[2]
    kch = h // 128
    P32 = NT * NH
    w1s = np.ascontiguousarray(
        W1.reshape(MLP, kch, 128).transpose(2, 1, 0)).astype(np.float32)
    cw = np.zeros((MLP, MLP + 2 * P32 * NT + 2), dtype=np.float32)
    cw[:, 0:MLP] = W2.T
    # zero-padded per-tile head blocks: block t covers psum rows 8t..8t+8
    for t in range(NT):
        cw[:, MLP + P32 * t + NH * t:MLP + P32 * t + NH * (t + 1)] = Wq.T
        base_v = MLP + P32 * NT
        cw[:, base_v + P32 * t + NH * t:base_v + P32 * t + NH * (t + 1)] = Wv.T
    cw[:, MLP + 2 * P32 * NT] = b1
    cw[:, MLP + 2 * P32 * NT + 1] = b2
    pos = np.arange(S, dtype=np.float32)
    maskadd = np.where(mask == 0, np.float32(-1e9), np.float32(0.0))  # (B,S)

    if KERNEL_DT == "bf16":
        import ml_dtypes
        mmdt = ml_dtypes.bfloat16
    else:
        mmdt = np.float32
    w1s = w1s.astype(mmdt)

    in_maps = []
    for c in range(NCORES):
        sl = slice(c * S_SHARD, (c + 1) * S_SHARD)
        xt = np.ascontiguousarray(
            x[:, sl, :].astype(mmdt).transpose(2, 0, 1).reshape(h, TOK))
        # ca row 8t+h covers (batch tile t, head h)
        ca = np.empty((P32, 512 + 1), dtype=np.float32)
        add_ths = (pos_w.astype(np.float32)[None, :, None]
                   * pos[sl][None, None, :]
                   + maskadd[:, None, sl])           # (B=NT, NH, 512)
        ca[:, 0:512] = add_ths.reshape(P32, 512)
        ca[:, 512] = np.tile(bv, NT)
        in_maps.append({"xt": xt, "w1s": w1s, "cw": cw,
                        "cwr": np.ascontiguousarray(cw[:, 0:MLP + 2 * P32 * NT]),
                        "ca": ca})
    return in_maps


def merge_stats(stats_all, bias):
    """stats_all: (NCORES, 32, 3), row 8t+h = (batch t, head h) with
    [-m, Z, W] -> (B, 1) output."""
    st = np.asarray(stats_all, dtype=np.float64).reshape(NCORES, NT, NH, 3)
    m = -st[..., 0]          # (C, B, NH)
    Z = st[..., 1]
    W = st[..., 2]
    M = m.max(axis=0)        # (B, NH)
    alpha = np.exp(m - M[None])
    Zg = (alpha * Z).sum(axis=0)
    Wg = (alpha * W).sum(axis=0)
    out = (Wg / Zg).sum(axis=1)          # (B,)
    return (out[:, None] + np.float64(bias.reshape(1)[0])).astype(np.float32)


def kernel(x, mask, W1, b1, W2, b2, Wq, Wv, bv, pos_w, bias, _trace=False):
    from concourse.bass_utils import run_bass_kernel_spmd

    x = np.asarray(x, dtype=np.float32)
    in_maps = make_core_inputs(x, np.asarray(mask), *(np.asarray(a) for a in
                               (W1, b1, W2, b2, Wq, Wv, bv, pos_w, bias)))
    nc = get_nc()
    res = run_bass_kernel_spmd(nc, in_maps, core_ids=list(range(NCORES)),
                               trace=_trace)
    stats_all = np.stack([r["stats"] for r in res.results])  # (C, NH, NT, 3)
    out = merge_stats(stats_all, np.asarray(bias))
    if _trace:
        kernel.last_result = res
    return out



# revision 4
# speedup vs baseline: 1.6871x; 1.6871x over previous
"""Trainium2 Bass kernel for nn_AttentionProbe_80891414053184.

Math (reference):
    y  = relu(x @ W1.T + b1)            # (B,S,H) -> (B,S,128)
    y2 = relu(y @ W2.T + b2)            # (B,S,128)
    l  = y2 @ Wq.T + pos*pos_w  (+mask) # (B,S,8) logits
    p  = softmax(l, axis=S)
    v  = y2 @ Wv.T + bv
    out[b] = sum_{s,h} p*v + bias       # (B,1)

Strategy: sequence-parallel over 8 cores (512 positions x 4 batches = 2048
tokens per core).  Each core streams its x-shard quantized to fp8-e4m3 on the
host (pre-transposed to (H, tokens) so the contraction dim lands on SBUF
partitions), runs layer 1 as DoubleRow fp8 matmuls (2 contraction rows per
PE cycle), the MLP tail + head projections in f32r, and emits per-(batch,
head) partial softmax stats (-max, Z=sum exp, W=sum exp*v).  The host merges
the 8 partial stats with the standard online-softmax combine.

fp8 numerics: x ~ N(0,1) quantizes to e4m3 directly.  W1 values (~1/64) sit
in e4m3's subnormal range, so the host scales W1 by 64 before quantizing and
folds the 1/64 back into W2 (relu is positively homogeneous, so
relu(z)/64 == relu(z/64) with b1 scaled by 64 on the psum side).  Measured
end-to-end error of this scheme vs the fp32 reference: ~3.4e-3.

The whole fp8 x-shard (8 MB) fits in SBUF (64 KB/partition), so all x DMAs
are issued up front with no buffer recycling -- the DMA stream free-runs at
HBM rate while the PE consumes groups in order.
"""

import os

import numpy as np

# Problem dims (hardcoded per harness contract).
B, S, H = 4, 4096, 4096
MLP, NH = 128, 8
NCORES = 8
S_SHARD = S // NCORES        # 512 seq positions per core
TOK = B * S_SHARD            # 2048 tokens per core
NT = TOK // 512              # 4 token tiles of 512 (= one batch each)
KCH = H // 128               # 32 contraction chunks of 128
GRP = int(os.environ.get("KERNEL_GRP", "4"))   # k-chunks per full x DMA
NGRP = KCH // GRP
P32 = NT * NH                # 32 packed (tile, head) lanes

_cache = {}


def _build_nc():
    import concourse.mybir as mybir
    import concourse.tile as tile
    from concourse import bacc
    from concourse.tile import add_dep_helper

    f32 = mybir.dt.float32
    f32r = mybir.dt.float32r
    fp8 = mybir.dt.float8e4

    # Bacc (not bare Bass): its finalize() runs move_matmul_waits_to_ldweights
    # and generate_event_semaphores, which split multi-sem waits to satisfy
    # TRN2's one-wait-per-instruction encoding limit.
    nc = bacc.Bacc()
    xt_d = nc.dram_tensor("xt", [H, TOK], fp8, kind="ExternalInput")
    w1_d = nc.dram_tensor("w1s", [128, KCH, MLP], fp8, kind="ExternalInput")
    # cwr: [w2t/64 | wq32 (4 x 32-wide zero-padded blocks) | wv32] f32r.
    # The zero-padded head blocks let all 4 token tiles accumulate their head
    # projections into ONE (32, 512) psum bank, so the whole softmax-stats
    # stage runs as single 32-lane instructions.
    cwr_d = nc.dram_tensor("cwr", [MLP, MLP + 2 * P32 * NT], f32r,
                           kind="ExternalInput")
    cb_d = nc.dram_tensor("cb", [MLP, 2], f32, kind="ExternalInput")  # 64*b1|b2
    # ca row 8t+h = (batch tile t, head h): [pos*pos_w + mask add | bv]
    ca_d = nc.dram_tensor("ca", [P32, 512 + 1], f32, kind="ExternalInput")
    st_d = nc.dram_tensor("stats", [P32, 3], f32, kind="ExternalOutput")

    AF = mybir.ActivationFunctionType
    AX = mybir.AxisListType
    OP = mybir.AluOpType
    PM = mybir.MatmulPerfMode.DoubleRow
    CQ = MLP                    # wq32 blocks start
    CV = MLP + P32 * NT         # wv32 blocks start

    with tile.TileContext(nc) as tc:
        with (
            tc.tile_pool(name="const", bufs=1) as const,
            tc.tile_pool(name="xp", bufs=NGRP - 1) as xp,
            tc.tile_pool(name="xlp", bufs=NT) as xlp,
            tc.tile_pool(name="yp", bufs=2) as yp,
            tc.tile_pool(name="y2p", bufs=2) as y2p,
            tc.tile_pool(name="smallp", bufs=1) as smallp,
            tc.tile_pool(name="statsp", bufs=1) as statsp,
            tc.tile_pool(name="ps_y", bufs=4, space="PSUM") as ps_y,
            tc.tile_pool(name="ps_y2", bufs=2, space="PSUM") as ps_y2,
            tc.tile_pool(name="ps_q", bufs=1, space="PSUM") as ps_q,
            tc.tile_pool(name="ps_v", bufs=1, space="PSUM") as ps_v,
        ):
            w1_sb = const.tile([128, KCH, MLP], fp8)
            # First pair-group gates the first matmuls; issue before x group 0
            # so only ~32 KB + 1 MB must land before the PE starts.
            nc.sync.dma_start(out=w1_sb[:, 0:GRP, :], in_=w1_d[:, 0:GRP, :])

            x_sb = []
            cwr_sb = cb_sb = ca_sb = None
            for g in range(NGRP - 1):
                xg = xp.tile([128, GRP, TOK], fp8, tag="x", name=f"x_sb{g}")
                nc.sync.dma_start(
                    out=xg[:],
                    in_=xt_d[g * GRP * 128:(g + 1) * GRP * 128, :].rearrange(
                        "(a p) n -> p a n", p=128))
                x_sb.append(xg)
                if g == 0:
                    # Remaining consts land while group 0 is being consumed.
                    nc.sync.dma_start(out=w1_sb[:, GRP:KCH, :],
                                      in_=w1_d[:, GRP:KCH, :])
                    cwr_sb = const.tile([MLP, MLP + 2 * P32 * NT], f32r)
                    nc.sync.dma_start(out=cwr_sb[:], in_=cwr_d[:])
                    cb_sb = const.tile([MLP, 2], f32)
                    nc.sync.dma_start(out=cb_sb[:], in_=cb_d[:])
                    ca_sb = const.tile([P32, 512 + 1], f32)
                    nc.sync.dma_start(out=ca_sb[:], in_=ca_d[:])
            # Last k-group arrives per token tile so tile t's MLP tail starts
            # while tile t+1's final columns are still streaming.
            xl_sb = []
            for t in range(NT):
                xl = xlp.tile([128, GRP, 512], fp8, tag="xl", name=f"xl{t}")
                nc.sync.dma_start(
                    out=xl[:],
                    in_=xt_d[(NGRP - 1) * GRP * 128:KCH * 128,
                             t * 512:(t + 1) * 512].rearrange(
                                 "(a p) n -> p a n", p=128))
                xl_sb.append(xl)

            stats_sb = statsp.tile([P32, 3], f32)

            # Warmup: PE observes the w1 first-group DMA lane before the real
            # matmuls so steady-state instructions carry at most one new wait.
            warm_ps = ps_y2.tile([128, 512], f32, tag="y2", name="warm_ps")
            warm_pe = nc.tensor.matmul(warm_ps[0:32, 0:64], w1_sb[:, 0:2, 0:32],
                                       w1_sb[:, 0:2, 0:64],
                                       start=True, stop=True, perf_mode=PM)

            # Layer 1: yT[t] (128, 512) += (64*W1T)_pair.T @ x_pair, DoubleRow
            # fp8 (2 contraction rows per cycle), k-accumulated over 16 pairs.
            psum_y = [ps_y.tile([128, 512], f32, tag="y", name=f"y_ps{t}")
                      for t in range(NT)]
            for g in range(NGRP):
                for kk in range(0, GRP, 2):
                    k = g * GRP + kk
                    for t in range(NT):
                        rhs = (x_sb[g][:, kk:kk + 2, t * 512:(t + 1) * 512]
                               if g < NGRP - 1 else xl_sb[t][:, kk:kk + 2, :])
                        mm = nc.tensor.matmul(
                            psum_y[t][:],
                            w1_sb[:, k:k + 2, :],
                            rhs,
                            start=(k == 0),
                            stop=(k == KCH - 2),
                            perf_mode=PM)
                        if k == 0 and t == 0:
                            add_dep_helper(mm.ins, warm_pe.ins, sync=False,
                                           reason="warmup before first mm")

            # cw/ca lane warmups (before their first real consumers in the
            # tail).
            warm_ps2 = ps_y2.tile([128, 512], f32, tag="y2", name="warm_ps2")
            nc.tensor.matmul(warm_ps2[0:NH, 0:NH], cwr_sb[:, 0:NH],
                             cwr_sb[:, 0:NH], start=True, stop=True)
            warm_act = const.tile([MLP, 1], f32)
            nc.scalar.copy(out=warm_act[:], in_=cb_sb[:, 0:1])
            warm_act8 = const.tile([P32, 1], f32)
            nc.scalar.copy(out=warm_act8[:], in_=ca_sb[:, 512:513])
            warm_dve = const.tile([P32, 1], f32)
            nc.vector.tensor_copy(out=warm_dve[:], in_=ca_sb[:, 0:1])

            q32_ps = ps_q.tile([P32, 512], f32, tag="q", name="q32_ps")
            v32_ps = ps_v.tile([P32, 512], f32, tag="v", name="v32_ps")
            for t in range(NT):
                y_sb = yp.tile([128, 512], f32r, tag="ysb", name=f"y_sb{t}")
                # y_sb = relu(psum + 64*b1) = 64*y; the 1/64 is folded into
                # cwr's W2 block.  relu on DVE (add+max) keeps ACT free for
                # relu2/exp in the tail.
                nc.vector.tensor_scalar(out=y_sb[:], in0=psum_y[t][:],
                                        scalar1=cb_sb[:, 0:1],
                                        scalar2=0.0, op0=OP.add, op1=OP.max)
                y2_ps = ps_y2.tile([128, 512], f32, tag="y2", name=f"y2_ps{t}")
                nc.tensor.matmul(y2_ps[:], cwr_sb[:, 0:MLP], y_sb[:],
                                 start=True, stop=True)
                y2_sb = y2p.tile([128, 512], f32r, tag="y2sb", name=f"y2_sb{t}")
                nc.scalar.activation(out=y2_sb[:], in_=y2_ps[:], func=AF.Relu,
                                     bias=cb_sb[:, 1:2], scale=1.0)
                # Head projections: the (128, 32) weight block for tile t is
                # zero outside rows 8t..8t+8, so accumulating all 4 tiles into
                # one (32, 512) bank packs q/v as (tile, head) x seq lanes.
                nc.tensor.matmul(q32_ps[:],
                                 cwr_sb[:, CQ + P32 * t:CQ + P32 * (t + 1)],
                                 y2_sb[:], start=(t == 0), stop=(t == NT - 1))
                nc.tensor.matmul(v32_ps[:],
                                 cwr_sb[:, CV + P32 * t:CV + P32 * (t + 1)],
                                 y2_sb[:], start=(t == 0), stop=(t == NT - 1))

            l_sb = smallp.tile([P32, 512], f32, tag="l", name="l_sb")
            nc.vector.tensor_add(out=l_sb[:], in0=q32_ps[:],
                                 in1=ca_sb[:, 0:512])
            # stats[:, 0] = -max_s l
            nc.vector.tensor_reduce(out=stats_sb[:, 0:1], in_=l_sb[:],
                                    axis=AX.X, op=OP.max, negate=True)
            v_sb = smallp.tile([P32, 512], f32, tag="vs", name="v_sb")
            nc.vector.tensor_scalar_add(out=v_sb[:], in0=v32_ps[:],
                                        scalar1=ca_sb[:, 512:513])
            e_sb = smallp.tile([P32, 512], f32, tag="e", name="e_sb")
            # e = exp(l - max); stats[:, 1] = Z = sum e
            nc.scalar.activation(out=e_sb[:], in_=l_sb[:], func=AF.Exp,
                                 bias=stats_sb[:, 0:1], scale=1.0,
                                 accum_out=stats_sb[:, 1:2])
            ev_sb = smallp.tile([P32, 512], f32, tag="ev", name="ev_sb")
            nc.vector.tensor_mul(out=ev_sb[:], in0=e_sb[:], in1=v_sb[:])
            # stats[:, 2] = W = sum e*v
            nc.vector.tensor_reduce(out=stats_sb[:, 2:3], in_=ev_sb[:],
                                    axis=AX.X, op=OP.add)

            nc.sync.dma_start(out=st_d[:], in_=stats_sb[:])

    nc.finalize()
    return nc


def get_nc():
    if "nc" not in _cache:
        _cache["nc"] = _build_nc()
    return _cache["nc"]


def make_core_inputs(x, mask, W1, b1, W2, b2, Wq, Wv, bv, pos_w, bias):
    """Host-side shard + transpose + fp8 quantize. Returns 8 in_maps."""
    import ml_dtypes
    FP8 = ml_dtypes.float8_e4m3

    # W1 scaled by 64 so its values quantize in e4m3's normal range; the
    # matching 1/64 is folded into W2 below (exact: power of two).
    w1s = np.ascontiguousarray(
        (W1.astype(np.float32) * 64.0).reshape(MLP, KCH, 128)
        .transpose(2, 1, 0)).astype(FP8)

    cwr = np.zeros((MLP, MLP + 2 * P32 * NT), dtype=np.float32)
    cwr[:, 0:MLP] = W2.T / 64.0
    # zero-padded per-tile head blocks: block t covers psum rows 8t..8t+8
    for t in range(NT):
        cwr[:, MLP + P32 * t + NH * t:MLP + P32 * t + NH * (t + 1)] = Wq.T
        base_v = MLP + P32 * NT
        cwr[:, base_v + P32 * t + NH * t:base_v + P32 * t + NH * (t + 1)] = Wv.T
    cb = np.stack([b1.astype(np.float32) * 64.0,
                   b2.astype(np.float32)], axis=1)
    cb = np.ascontiguousarray(cb, dtype=np.float32)

    pos = np.arange(S, dtype=np.float32)
    maskadd = np.where(mask == 0, np.float32(-1e9), np.float32(0.0))  # (B,S)

    in_maps = []
    for c in range(NCORES):
        sl = slice(c * S_SHARD, (c + 1) * S_SHARD)
        xt = np.ascontiguousarray(
            x[:, sl, :].transpose(2, 0, 1).reshape(H, TOK)).astype(FP8)
        ca = np.empty((P32, 512 + 1), dtype=np.float32)
        add_ths = (pos_w.astype(np.float32)[None, :, None]
                   * pos[sl][None, None, :]
                   + maskadd[:, None, sl])           # (B=NT, NH, 512)
        ca[:, 0:512] = add_ths.reshape(P32, 512)
        ca[:, 512] = np.tile(bv, NT)
        in_maps.append({"xt": xt, "w1s": w1s, "cwr": cwr, "cb": cb, "ca": ca})
    return in_maps


def merge_stats(stats_all, bias):
    """stats_all: (NCORES, 32, 3), row 8t+h = (batch t, head h) with
    [-m, Z, W] -> (B, 1) output."""
    st = np.asarray(stats_all, dtype=np.float64).reshape(NCORES, NT, NH, 3)
    m = -st[..., 0]          # (C, B, NH)
    Z = st[..., 1]
    W = st[..., 2]
    M = m.max(axis=0)        # (B, NH)
    alpha = np.exp(m - M[None])
    Zg = (alpha * Z).sum(axis=0)
    Wg = (alpha * W).sum(axis=0)
    out = (Wg / Zg).sum(axis=1)          # (B,)
    return (out[:, None] + np.float64(bias.reshape(1)[0])).astype(np.float32)


def kernel(x, mask, W1, b1, W2, b2, Wq, Wv, bv, pos_w, bias, _trace=False):
    from concourse.bass_utils import run_bass_kernel_spmd

    x = np.asarray(x, dtype=np.float32)
    in_maps = make_core_inputs(x, np.asarray(mask), *(np.asarray(a) for a in
                               (W1, b1, W2, b2, Wq, Wv, bv, pos_w, bias)))
    nc = get_nc()
    res = run_bass_kernel_spmd(nc, in_maps, core_ids=list(range(NCORES)),
                               trace=_trace)
    stats_all = np.stack([r["stats"] for r in res.results])  # (C, 32, 3)
    out = merge_stats(stats_all, np.asarray(bias))
    if _trace:
        kernel.last_result = res
    return out
